# revision 25
# baseline (speedup 1.0000x reference)
"""Trainium2 Bass kernel for nn_DTIModel (DTI pairwise message passing), v2.

Sharding: data-parallel over batch B=8 across 8 NeuronCores (one batch element
per core, weights replicated). The [Np*Nd, H] pairwise tensor never touches
DRAM: produced, attended (D=3) and reduced on-chip.

v2 design (vs the bf16 v1 baseline):
  * Tiles processed in PAIRS (G=2 drugs -> [128, 1024] elementwise ops) to
    amortize fixed per-op costs; the 96 drug-tiles are fully independent.
  * All pairwise matmuls are fp8e4 DoubleRow (0.5 cyc/row): the second
    contraction block of each DR pair carries either real data (virtual
    m1 = m0+g0, m2 = m1+g1 summed inside PSUM) or a constant block that adds
    the bias: stationary row 0 holds 16*b over a moving ONES block.
  * Weights scaled x16 (alpha-matrices x64) to dodge fp8e4 subnormals;
    descaling folded into the evacuation scale and the STT scalar.
  * No Pool m-adds: m1/m2 are never materialized (PE pairs (m0|g0), (g0|g1)).
  * Per depth: one Act evacuation of ps_y (scale 1/16 -> bf16) + one DVE
    STT g = (ps_B * 1/64) * y -> fp8. Assignment of the evacuations between
    Act and DVE is tuned so both engines carry ~equal load.

Math identical to v1 (see reference): per pair x = lrelu(pv_i + dv_j):
  loop d: y = x@Wd + bd; a = y@att_d + attb_d; x += a*y
  s += relu((m1 + g1 + 2*g2) @ W1a + b1a);  head MLP on s, u_p, u_d.
"""

import numpy as np

import concourse.bass as bass
import concourse.mybir as mybir
import concourse.tile as tile
from concourse.masks import make_identity

F32 = mybir.dt.float32
BF16 = mybir.dt.bfloat16
F8 = mybir.dt.float8e4
ALU = mybir.AluOpType
ACTF = mybir.ActivationFunctionType
DRM = mybir.MatmulPerfMode.DoubleRow

H = 128
NP = 512
ND = 96
NPAIR = NP * ND  # 49152
N_CORES = 8
G = 2            # drugs per group (overridable via knobs["G"])
NG = ND // G
GF = G * NP

SW = 16.0        # W-side fp8 scale
SR = 64.0        # alpha-side fp8 scale

_INPUT_SPECS = {
    "protein_features": ([NP, H], True),
    "drug_features": ([ND, H], True),
    "pu_mask": ([NP], True),
    "du_mask": ([ND], True),
    "d_W": ([H, H], False), "d_b": ([H], False),
    "p_W": ([H, H], False), "p_b": ([H], False),
    "Wv_p": ([H, H], False), "Wv_d": ([H, H], False), "Wv_b": ([H], False),
    "att_W": ([3, H, 1], False), "att_b": ([3, 1], False),
    "Wvs_W": ([3, H, H], False), "Wvs_b": ([3, H], False),
    "Wu_W": ([2 * H, H], False), "Wu_b": ([H], False),
    "W1a_W": ([H, H], False), "W1a_b": ([H], False),
    "W1b_W": ([H, H], False), "W1b_b": ([H], False),
    "W2a_W": ([H, 2 * H], False), "W2a_b": ([2 * H], False),
    "W2b_W": ([2 * H, H], False), "W2b_b": ([H], False),
    "W3_W": ([H, H], False), "W3_b": ([H], False),
    "Wu1a_W": ([2 * H, H], False), "Wu1a_b": ([H], False),
    "Wu1b_W": ([H, H], False), "Wu1b_b": ([H], False),
    "W5_W": ([H, 1], False), "W5_b": ([1], False),
}

_LCNT = [0]


def _lrelu_col(nc, pool, psum_col, bias_col, slope):
    _LCNT[0] += 1
    o = pool.tile([H, 1], F32, tag="lo%d" % _LCNT[0])
    nc.scalar.activation(o, psum_col, ACTF.Prelu, bias=bias_col, alpha=slope)
    return o


def _legalize_multiwaits(nc):
    """Split multi-wait sync_infos (walrus supports one wait per instr)."""
    uid = [0]
    for fn in nc.m.functions:
        for blk in fn.blocks:
            out = []
            changed = False
            for inst in blk.instructions:
                si = inst.sync_info
                waits = list(si.on_wait) if si is not None else []
                if len(waits) > 1:
                    for w in waits[:-1]:
                        uid[0] += 1
                        ev = mybir.InstEventSemaphore(
                            name="I-mwsplit-%d" % uid[0], ins=[], outs=[],
                            engine=inst.engine)
                        ev.sync_info = mybir.SyncInfo(on_wait=[w], on_update=[])
                        out.append(ev)
                    inst.sync_info = mybir.SyncInfo(
                        on_wait=[waits[-1]], on_update=list(si.on_update))
                    changed = True
                out.append(inst)
            if changed:
                blk.instructions = out


def build_bass(knobs=None):
    nc = bass.Bass()
    din = {}
    for name, (shape, _) in _INPUT_SPECS.items():
        din[name] = nc.declare_dram_parameter(name, list(shape), F32, isOutput=False)
    dout = nc.declare_dram_parameter("out", [1, 1], F32, isOutput=True)

    with tile.TileContext(nc) as tc:
        _emit(nc, tc, din, dout, knobs or {})
    _legalize_multiwaits(nc)
    return nc


def _emit(nc, tc, din, dout, knobs):
    import contextlib

    # evac assignment: which engine evacuates ps_y per depth; 'split' puts
    # cols [0:split_at) on Act and the rest on DVE.
    global G, NG, GF
    G = knobs.get("G", 1)
    NG = ND // G
    GF = G * NP
    evac_eng = knobs.get("evac_eng", ("act", "act", "alt"))
    alt_mod = knobs.get("alt_mod", 6)
    zs_mod = knobs.get("zs_mod", 8)
    split_at = knobs.get("split_at", 192)          # of GF=1024
    zs_eng = knobs.get("zs_eng", "act")
    y_bufs = knobs.get("y_bufs", 2)
    b_bufs = knobs.get("b_bufs", 1)
    s4_bufs = knobs.get("s4_bufs", 1)
    mega_bufs = knobs.get("mega_bufs", 3)

    ctx = contextlib.ExitStack()
    with ctx:
        const = ctx.enter_context(tc.tile_pool(name="const", bufs=1))
        setup = ctx.enter_context(tc.tile_pool(name="setup", bufs=2))
        spsum_cm = tc.tile_pool(name="spsum", bufs=1, space="PSUM")
        spsum = spsum_cm.__enter__()

        _uid = [0]

        def _tag(p):
            _uid[0] += 1
            return "%s%d" % (p, _uid[0])

        def load(name, src=None, shape=None, q=None):
            src = src if src is not None else din[name]
            if not isinstance(src, bass.AP):
                src = src[:]
            t = const.tile(shape or list(src.shape), F32, tag=_tag("w"))
            (q or nc.sync).dma_start(out=t, in_=src)
            return t

        def load_col(src, n=H, q=None):
            if not isinstance(src, bass.AP):
                src = src[:]
            t = const.tile([n, 1], F32, tag=_tag("b"))
            (q or nc.sync).dma_start(out=t, in_=src.rearrange("(n o) -> n o", o=1))
            return t

        def load_row(src, n=H, q=None):
            if not isinstance(src, bass.AP):
                src = src[:]
            t = const.tile([1, n], F32, tag=_tag("r"))
            (q or nc.sync).dma_start(out=t, in_=src.rearrange("(o n) -> o n", o=1))
            return t

        # ---------------- setup emission order ---------------------------
        # Pool first: identity (transposes need it) before any gpsimd DMA.
        ident = const.tile([H, H], F32)
        make_identity(nc, ident)
        ones_row = const.tile([1, H], F32)
        nc.vector.memset(ones_row, 1.0)
        ones8 = const.tile([H, H], F8)
        nc.vector.memset(ones8, 1.0)

        # One shared HWDGE line: minimize pre-loop DMA count (merge the [3,..]
        # weight stacks into single DMAs) and order strictly by need.
        nat4 = setup.tile([H, 4 * H], F32, tag="nat4")
        nc.sync.dma_start(
            out=nat4[:, :].rearrange("p (c m) -> p c m", c=4),
            in_=din["protein_features"][:].rearrange("(c p) m -> p c m", c=4))
        p_W = load("p_W")
        p_b = load_col(din["p_b"])
        pu_row = load_row(din["pu_mask"], n=NP)
        natd = setup.tile([H, H], F32, tag="natd")
        nc.sync.dma_start(out=natd[0:ND, :], in_=din["drug_features"][:, :])
        d_W = load("d_W")
        d_b = load_col(din["d_b"])
        du_row = load_row(din["du_mask"], n=ND)
        Wv_p = load("Wv_p")
        Wv_d = load("Wv_d")
        Wv_b = load_col(din["Wv_b"])
        nat_pt = [nat4[:, t * H:(t + 1) * H] for t in range(4)]

        # Stationary sources on the gpsimd queue (separate SWDGE line on the
        # idle Pool engine), merged into one DMA per parameter stack.
        gq = nc.gpsimd
        Wvs3 = const.tile([H, 3 * H], F32)
        gq.dma_start(out=Wvs3[:, :].rearrange("p (d m) -> p d m", d=3),
                     in_=din["Wvs_W"][:].rearrange("d p m -> p d m"))
        attW3 = const.tile([H, 3], F32)
        gq.dma_start(out=attW3[:, :].rearrange("p (d o) -> p d o", d=3),
                     in_=din["att_W"][:].rearrange("d p o -> p d o"))
        bcol3 = const.tile([H, 3], F32)
        gq.dma_start(out=bcol3, in_=din["Wvs_b"][:].rearrange("d p -> p d"))
        attb3 = const.tile([1, 3], F32)
        gq.dma_start(out=attb3, in_=din["att_b"][:].rearrange("d o -> o d"))
        W1a = load("W1a_W", q=gq)
        W1a_b = load_col(din["W1a_b"], q=gq)
        Wvs = [Wvs3[:, d * H:(d + 1) * H] for d in range(3)]
        attW = [attW3[:, d:d + 1] for d in range(3)]
        bcol = [bcol3[:, d:d + 1] for d in range(3)]
        att_b = [attb3[:, d:d + 1] for d in range(3)]

        # ---------------- transposes: PTt [H,NP], DRt [H,ND] -------------
        PTt = const.tile([H, NP], F32)
        for t in range(4):
            ps = spsum.tile([H, H], F32, tag="tp%d" % (t % 2))
            nc.tensor.transpose(ps, nat_pt[t], ident)
            nc.scalar.activation(PTt[:, t * H:(t + 1) * H], ps, ACTF.Copy)
        DRt = const.tile([H, ND], F32)
        psd = spsum.tile([H, H], F32, tag="tp0")
        nc.tensor.transpose(psd[:, 0:ND], natd[0:ND, :], ident[0:ND, 0:ND])
        nc.scalar.activation(DRt, psd[:, 0:ND], ACTF.Copy)

        # ---------------- stage 1: features ------------------------------
        def feat(WT, Xt, b_col, mask_row, n):
            ps = spsum.tile([H, NP], F32, tag="s1p")
            for c0 in range(0, n, H):
                c1 = min(c0 + H, n)
                nc.tensor.matmul(ps[:, c0:c1], WT, Xt[:, c0:c1],
                                 start=True, stop=True)
            l = setup.tile([H, NP], F32, tag="s1l")
            nc.scalar.activation(l[:, 0:n], ps[:, 0:n], ACTF.Prelu,
                                 bias=b_col, alpha=0.1)
            pm = spsum.tile([H, NP], F32, tag="s1m")
            nc.tensor.matmul(pm[:, 0:n], ones_row, mask_row, start=True, stop=True)
            f = setup.tile([H, NP], F32, tag="s1f")
            nc.vector.scalar_tensor_tensor(
                f[:, 0:n], l[:, 0:n], 1.0, pm[:, 0:n], ALU.mult, ALU.mult)
            return f

        prot = feat(p_W, PTt, p_b, pu_row, NP)      # [128, 512] f32
        drug = feat(d_W, DRt, d_b, du_row, ND)      # [128, 96]

        u_p = const.tile([H, 1], F32)
        nc.vector.tensor_reduce(u_p, prot[:, 0:NP], mybir.AxisListType.X, ALU.add)
        u_d = const.tile([H, 1], F32)
        nc.vector.tensor_reduce(u_d, drug[:, 0:ND], mybir.AxisListType.X, ALU.add)

        ps_pv = spsum.tile([H, NP], F32, tag="s1p")
        nc.tensor.matmul(ps_pv, Wv_p, prot[:, 0:NP], start=True, stop=True)
        pv = const.tile([H, NP], F32)
        nc.scalar.activation(pv, ps_pv, ACTF.Copy)

        ps_dv = spsum.tile([H, ND], F32, tag="s1m")
        nc.tensor.matmul(ps_dv, Wv_d, drug[:, 0:ND], start=True, stop=True)
        dvf = const.tile([H, ND], F32)
        nc.scalar.activation(dvf, ps_dv, ACTF.Identity, bias=Wv_b)

        # ---------------- fp8 stationaries [128, 256] --------------------
        # statY_d  = [b-row-block | 16*Wvs_d]     pairs rhs (ONES | m0)
        # statYz_d = [0 | 16*Wvs_d]               pairs rhs (m0 | g0) etc.
        # statB_d  = [c-row-block | SRd*R_d]      pairs rhs (ONES | m0)
        # statBz_d = [0 | SRd*R_d]
        # statV    = [16*W1a | 16*W1a]
        statY, statYz, statB, statBz = [], [], [], []
        c_col, b_ev = [], []
        for d in range(3):
            srd = SR * (2.0 if d == 2 else 1.0)
            # A_d = Wvs_d @ att_d  via transpose then matmul (as in v1)
            psT = spsum.tile([H, H], F32, tag="tps")
            nc.tensor.transpose(psT, Wvs[d], ident)
            WvsT = setup.tile([H, H], F32, tag="wvsT")
            nc.vector.tensor_copy(WvsT, psT)
            psA = spsum.tile([H, 1], F32, tag="smu")
            nc.tensor.matmul(psA, WvsT, attW[d], start=True, stop=True)
            A_col = setup.tile([H, 1], F32, tag="acol")
            nc.vector.tensor_scalar(A_col, psA, srd, None, ALU.mult)
            R8 = setup.tile([H, H], F8, tag="r8")
            nc.vector.tensor_scalar(R8, ones8, A_col, None, ALU.mult)

            # c_d = b_d.att_d + att_b_d; as [128,1] col times srd (for the
            # STT scalar-add), and b_d/SR col (for the evac bias).
            psc = spsum.tile([1, 1], F32, tag="psc")
            nc.tensor.matmul(psc, bcol[d], attW[d], start=True, stop=True)
            c1 = setup.tile([1, 1], F32, tag="c1")
            nc.vector.tensor_scalar(c1, psc, att_b[d], srd, ALU.add, ALU.mult)
            pscb = spsum.tile([H, 1], F32, tag="smu")
            nc.tensor.matmul(pscb, ones_row, c1, start=True, stop=True)
            cc = const.tile([H, 1], F32, tag=_tag("cc"))
            nc.scalar.activation(cc, pscb, ACTF.Copy)
            c_col.append(cc)
            bb = const.tile([H, 1], F32, tag=_tag("bb"))
            nc.vector.tensor_scalar(bb, bcol[d], 1.0 / SR, None, ALU.mult)
            b_ev.append(bb)

            sy = const.tile([H, 2 * H], F8, tag=_tag("st"))
            nc.gpsimd.memset(sy[:, 0:H], 0.0)
            nc.vector.tensor_scalar(sy[:, H:2 * H], Wvs[d], SW, None, ALU.mult)
            statY.append(sy)
            statYz.append(sy)

            sb = const.tile([H, 2 * H], F8, tag=_tag("st"))
            nc.gpsimd.memset(sb[:, 0:H], 0.0)
            nc.vector.tensor_copy(sb[:, H:2 * H], R8)
            statB.append(sb)
            statBz.append(sb)

        statV = const.tile([H, 2 * H], F8)
        nc.vector.tensor_scalar(statV[:, 0:H], W1a, SW, None, ALU.mult)
        nc.vector.tensor_scalar(statV[:, H:2 * H], W1a, SW, None, ALU.mult)

        # for d2 second block: statY2p = [16W2 | 16W2], statB2p = [R2' | R2']
        statY2p = const.tile([H, 2 * H], F8)
        nc.vector.tensor_scalar(statY2p[:, 0:H], Wvs[2], SW, None, ALU.mult)
        nc.vector.tensor_scalar(statY2p[:, H:2 * H], Wvs[2], SW, None, ALU.mult)
        statB2p = const.tile([H, 2 * H], F8)
        nc.vector.tensor_copy(statB2p[:, 0:H], statBz[2][:, H:2 * H])
        nc.vector.tensor_copy(statB2p[:, H:2 * H], statBz[2][:, H:2 * H])

        b1a16 = const.tile([H, 1], F32)
        nc.vector.tensor_scalar(b1a16, W1a_b, SW, None, ALU.mult)
        zeros_bf = const.tile([H, NP], BF16)
        nc.gpsimd.memset(zeros_bf, 0.0)

        sacc = const.tile([H, NG], F32)

        spsum_cm.__exit__(None, None, None)

        # ---------------- pairwise main loop: 48 groups of 2 drugs -------
        # mega slot layout (fp8, per group): [ONES | m0 | g0 | g1 | g2],
        # each [128, GF]. DR rhs for tile t of the pair: view two adjacent
        # slots as [128, 2, GF] and take [:, :, t*NP:(t+1)*NP].
        SLOT = GF

        def pair_ap(mega, s, t):
            v = mega[:, s * SLOT:(s + 2) * SLOT]
            v = v.rearrange("p (a b) -> p a b", a=2)
            return v[:, :, t * NP:(t + 1) * NP]

        # PSUM plan (8 banks of [128,512]f32), all rotations benign:
        #   psY: y [128,1024] bufs=2 (4 banks)  y0/y1/y2 rotation
        #   psB: B [128,512] per-tile bufs=2 (2 banks)
        #   psS: s4 [128,1024] bufs=1 (2 banks)
        with tc.tile_pool(name="mega", bufs=mega_bufs) as megap, \
             tc.tile_pool(name="ysb", bufs=knobs.get("ysb_bufs", 2)) as ysbp, \
             tc.tile_pool(name="psY", bufs=knobs.get("psy_bufs", 2), space="PSUM") as psY, \
             tc.tile_pool(name="psBt", bufs=knobs.get("psb_bufs", 2), space="PSUM") as psBt, \
             tc.tile_pool(name="psS", bufs=knobs.get("pss_bufs", 1), space="PSUM") as psS:

            def st_m0(st):
                g = st["g"]
                mega = megap.tile([H, 5 * SLOT], F8, tag="mega")
                if g < mega_bufs:
                    nc.gpsimd.memset(mega[:, 0:SLOT], 1.0)  # ONES slot, once/buf
                st["mega"] = mega
                for t in range(G):
                    j = g * G + t
                    nc.scalar.activation(
                        mega[:, SLOT + t * NP: SLOT + (t + 1) * NP], pv,
                        ACTF.Prelu, bias=dvf[:, j:j + 1], alpha=0.1)

            def st_pe0(st):
                mega = st["mega"]
                py = psY.tile([H, GF], F32, tag="y")
                pbs = []
                for t in range(G):
                    rhs = pair_ap(mega, 0, t)   # (ONES | m0)
                    nc.tensor.matmul(py[:, t * NP:(t + 1) * NP],
                                     statY[0][:, :].rearrange("p (a b) -> p a b", a=2),
                                     rhs, start=True, stop=True, perf_mode=DRM)
                    pb = psBt.tile([H, NP], F32, tag="b")
                    nc.tensor.matmul(pb,
                                     statB[0][:, :].rearrange("p (a b) -> p a b", a=2),
                                     rhs, start=True, stop=True, perf_mode=DRM)
                    pbs.append(pb)
                st["py"], st["pb"] = py, pbs

            def _evac(st, d):
                py = st["py"]
                y = ysbp.tile([H, GF], BF16, tag="y%d" % d)
                eng = evac_eng[d]
                if eng == "alt":
                    eng = "act" if (st["g"] % alt_mod) == 0 else "dve"
                if eng == "act":
                    nc.scalar.activation(y, py, ACTF.Identity, bias=b_ev[d],
                                         scale=1.0 / (SW * SR))
                else:
                    nc.vector.tensor_scalar(y, py, 1.0 / (SW * SR), b_ev[d],
                                            ALU.mult, ALU.add)
                st["y"] = y

            def _stt(st, d, slot):
                # g_d = (ps_B * 1/SR) * y -> fp8 into mega slot (per tile)
                mega = st["mega"]
                for t in range(G):
                    o = slice(slot * SLOT + t * NP, slot * SLOT + (t + 1) * NP)
                    nc.vector.scalar_tensor_tensor(
                        mega[:, o], st["pb"][t], c_col[d],
                        st["y"][:, t * NP:(t + 1) * NP], ALU.add, ALU.mult)

            def st_ev0(st):
                _evac(st, 0)

            def st_g0(st):
                _stt(st, 0, 2)

            def st_pe1(st):
                mega = st["mega"]
                py = psY.tile([H, GF], F32, tag="y")
                pbs = []
                for t in range(G):
                    r_om = pair_ap(mega, 0, t)   # (ONES | m0)
                    r_mg = pair_ap(mega, 1, t)   # (m0 | g0)
                    o = slice(t * NP, (t + 1) * NP)
                    nc.tensor.matmul(py[:, o], statY[1][:, :].rearrange("p (a b) -> p a b", a=2),
                                     r_om, start=True, stop=False, perf_mode=DRM)
                    nc.tensor.matmul(py[:, o], statYz[1][:, :].rearrange("p (a b) -> p a b", a=2),
                                     r_mg, start=False, stop=True, perf_mode=DRM)
                    pb = psBt.tile([H, NP], F32, tag="b")
                    nc.tensor.matmul(pb, statB[1][:, :].rearrange("p (a b) -> p a b", a=2),
                                     r_om, start=True, stop=False, perf_mode=DRM)
                    nc.tensor.matmul(pb, statBz[1][:, :].rearrange("p (a b) -> p a b", a=2),
                                     r_mg, start=False, stop=True, perf_mode=DRM)
                    pbs.append(pb)
                st["py"], st["pb"] = py, pbs

            def st_ev1(st):
                _evac(st, 1)

            def st_g1(st):
                _stt(st, 1, 3)

            def st_pe2(st):
                mega = st["mega"]
                py = psY.tile([H, GF], F32, tag="y")
                pbs = []
                for t in range(G):
                    r_om = pair_ap(mega, 0, t)   # (ONES | m0)
                    r_gg = pair_ap(mega, 2, t)   # (g0 | g1)
                    o = slice(t * NP, (t + 1) * NP)
                    nc.tensor.matmul(py[:, o], statY[2][:, :].rearrange("p (a b) -> p a b", a=2),
                                     r_om, start=True, stop=False, perf_mode=DRM)
                    nc.tensor.matmul(py[:, o], statY2p[:, :].rearrange("p (a b) -> p a b", a=2),
                                     r_gg, start=False, stop=True, perf_mode=DRM)
                    pb = psBt.tile([H, NP], F32, tag="b")
                    nc.tensor.matmul(pb, statB[2][:, :].rearrange("p (a b) -> p a b", a=2),
                                     r_om, start=True, stop=False, perf_mode=DRM)
                    nc.tensor.matmul(pb, statB2p[:, :].rearrange("p (a b) -> p a b", a=2),
                                     r_gg, start=False, stop=True, perf_mode=DRM)
                    pbs.append(pb)
                st["py"], st["pb"] = py, pbs

            def st_ev2(st):
                _evac(st, 2)

            def st_g2(st):
                _stt(st, 2, 4)

            def st_pe4(st):
                mega = st["mega"]
                p4 = psS.tile([H, GF], F32, tag="s4")
                for t in range(G):
                    r_mg = pair_ap(mega, 1, t)   # (m0 | g0)
                    r_gg2 = pair_ap(mega, 3, t)  # (g1 | g2)
                    o = slice(t * NP, (t + 1) * NP)
                    nc.tensor.matmul(p4[:, o], statV[:, :].rearrange("p (a b) -> p a b", a=2),
                                     r_mg, start=True, stop=False, perf_mode=DRM)
                    nc.tensor.matmul(p4[:, o], statV[:, :].rearrange("p (a b) -> p a b", a=2),
                                     r_gg2, start=False, stop=True, perf_mode=DRM)
                st["p4"] = p4

            def st_zs(st):
                g = st["g"]
                if (g % zs_mod) == 0:
                    zso = ysbp.tile([H, GF], F8, tag="zso")
                    nc.scalar.activation(zso, st["p4"], ACTF.Relu, bias=b1a16,
                                         scale=1.0,
                                         accum_out=sacc[:, g:g + 1])
                else:
                    zso = ysbp.tile([H, GF], BF16, tag="zsv")
                    nc.vector.scalar_tensor_tensor(
                        zso, st["p4"], b1a16, zeros_bf[:, 0:GF],
                        ALU.add, ALU.max, accum_out=sacc[:, g:g + 1])

            stages = [st_m0, st_pe0, st_ev0, st_g0, st_pe1, st_ev1, st_g1,
                      st_pe2, st_ev2, st_g2, st_pe4, st_zs]
            # slot offset of each stage within a group's schedule; groups are
            # spaced SPACING slots apart.
            OFS = knobs.get("ofs", [0, 0, 1, 1, 2, 2, 3, 3, 4, 4, 5, 6])
            SPACING = knobs.get("spacing", 4)
            NST = len(stages)
            live = {}
            total_slots = (NG - 1) * SPACING + OFS[-1] + 1
            # within a slot: oldest group (largest OFS) first; within a group
            # (equal OFS), ascending stage order.
            order = sorted(range(NST), key=lambda s: (-OFS[s], s))
            for slot in range(total_slots):
                for s in order:
                    rem = slot - OFS[s]
                    if rem < 0 or rem % SPACING != 0:
                        continue
                    g = rem // SPACING
                    if g < 0 or g >= NG:
                        continue
                    if s == 0:
                        live[g] = {"g": g}
                    stages[s](live[g])
                    if s == NST - 1:
                        del live[g]

        # ---------------- head (tiny, f32) -------------------------------
        W1b = load("W1b_W", q=gq); W3 = load("W3_W", q=gq)
        Wu1b = load("Wu1b_W"); W5 = load("W5_W")
        W2a_lo = load(None, src=din["W2a_W"][:, 0:H])
        W2a_hi = load(None, src=din["W2a_W"][:, H:2 * H])
        W2b_lo = load(None, src=din["W2b_W"][0:H, :])
        W2b_hi = load(None, src=din["W2b_W"][H:2 * H, :])
        Wu_lo = load(None, src=din["Wu_W"][0:H, :])
        Wu_hi = load(None, src=din["Wu_W"][H:2 * H, :])
        Wu1a_lo = load(None, src=din["Wu1a_W"][0:H, :])
        Wu1a_hi = load(None, src=din["Wu1a_W"][H:2 * H, :])
        Wu_b = load_col(din["Wu_b"]); W1b_b = load_col(din["W1b_b"])
        W2a_b_lo = load_col(din["W2a_b"][0:H]); W2a_b_hi = load_col(din["W2a_b"][H:2 * H])
        W2b_b = load_col(din["W2b_b"]); W3_b = load_col(din["W3_b"])
        Wu1a_b = load_col(din["Wu1a_b"]); Wu1b_b = load_col(din["Wu1b_b"])
        W5_b = load_col(din["W5_b"], n=1)

        with tc.tile_pool(name="head", bufs=1) as hp, \
             tc.tile_pool(name="hpsum", bufs=1, space="PSUM") as hps:
            ps_mu = hps.tile([H, 1], F32, tag="h1")
            nc.tensor.matmul(ps_mu, Wu_lo, u_d, start=True, stop=False)
            nc.tensor.matmul(ps_mu, Wu_hi, u_p, start=False, stop=True)
            m_u = _lrelu_col(nc, hp, ps_mu, Wu_b, 0.01)

            s_raw = hp.tile([H, 1], F32)
            nc.vector.tensor_reduce(s_raw, sacc, mybir.AxisListType.X, ALU.add)
            s_col = hp.tile([H, 1], F32)
            nc.vector.tensor_scalar(s_col, s_raw, 1.0 / SW, None, ALU.mult)

            ps_g1 = hps.tile([H, 1], F32, tag="h1")
            nc.tensor.matmul(ps_g1, W1b, s_col, start=True, stop=True)
            w1bbN = hp.tile([H, 1], F32)
            nc.vector.tensor_scalar(w1bbN, W1b_b, float(NPAIR), None, ALU.mult)
            g1 = hp.tile([H, 1], F32)
            nc.vector.tensor_scalar(g1, ps_g1, w1bbN, None, ALU.add)

            ps_lo = hps.tile([H, 1], F32, tag="h2")
            nc.tensor.matmul(ps_lo, W2a_lo, g1, start=True, stop=True)
            t_lo = _lrelu_col(nc, hp, ps_lo, W2a_b_lo, 0.1)
            ps_hi = hps.tile([H, 1], F32, tag="h3")
            nc.tensor.matmul(ps_hi, W2a_hi, g1, start=True, stop=True)
            t_hi = _lrelu_col(nc, hp, ps_hi, W2a_b_hi, 0.1)

            ps_g2 = hps.tile([H, 1], F32, tag="h4")
            nc.tensor.matmul(ps_g2, W2b_lo, t_lo, start=True, stop=False)
            nc.tensor.matmul(ps_g2, W2b_hi, t_hi, start=False, stop=True)
            g2 = hp.tile([H, 1], F32)
            nc.vector.tensor_scalar(g2, ps_g2, W2b_b, None, ALU.add)

            ps_g3 = hps.tile([H, 1], F32, tag="h5")
            nc.tensor.matmul(ps_g3, W3, g2, start=True, stop=True)
            g3 = _lrelu_col(nc, hp, ps_g3, W3_b, 0.1)

            ps_u = hps.tile([H, 1], F32, tag="h6")
            nc.tensor.matmul(ps_u, Wu1a_lo, m_u, start=True, stop=False)
            nc.tensor.matmul(ps_u, Wu1a_hi, g3, start=False, stop=True)
            h1 = _lrelu_col(nc, hp, ps_u, Wu1a_b, 0.1)

            ps_mu2 = hps.tile([H, 1], F32, tag="h7")
            nc.tensor.matmul(ps_mu2, Wu1b, h1, start=True, stop=True)
            mu = hp.tile([H, 1], F32)
            nc.vector.tensor_scalar(mu, ps_mu2, Wu1b_b, None, ALU.add)

            ps_o = hps.tile([1, 1], F32, tag="h8")
            nc.tensor.matmul(ps_o, W5, mu, start=True, stop=True)
            res = hp.tile([1, 1], F32)
            nc.vector.tensor_scalar(res, ps_o, W5_b, None, ALU.add)
            nc.sync.dma_start(out=dout[:, :], in_=res)


_CACHE = {}


def _get_nc():
    if "nc" not in _CACHE:
        _CACHE["nc"] = build_bass()
    return _CACHE["nc"]


def kernel(**inputs):
    from concourse.bass_utils import run_bass_kernel_spmd

    nc = _get_nc()
    per_core = {"protein_features", "drug_features", "pu_mask", "du_mask"}
    in_maps = []
    for b in range(N_CORES):
        m = {}
        for name in _INPUT_SPECS:
            arr = np.asarray(inputs[name], dtype=np.float32)
            m[name] = np.ascontiguousarray(arr[b]) if name in per_core else arr
        in_maps.append(m)
    res = run_bass_kernel_spmd(nc, in_maps, list(range(N_CORES)))
    out = np.stack([res.results[i]["out"].reshape(1) for i in range(N_CORES)])
    return out.astype(np.float32)


if __name__ == "__main__":
    nc = build_bass()
    print("build ok")


# revision 28
# speedup vs baseline: 1.5295x; 1.5295x over previous
"""Trainium2 Bass kernel for nn_DTIModel (DTI pairwise message passing), v2.

Sharding: data-parallel over batch B=8 across 8 NeuronCores (one batch element
per core, weights replicated). The [Np*Nd, H] pairwise tensor never touches
DRAM: produced, attended (D=3) and reduced on-chip.

v2 design (vs the bf16 v1 baseline):
  * Tiles processed in PAIRS (G=2 drugs -> [128, 1024] elementwise ops) to
    amortize fixed per-op costs; the 96 drug-tiles are fully independent.
  * All pairwise matmuls are fp8e4 DoubleRow (0.5 cyc/row): the second
    contraction block of each DR pair carries either real data (virtual
    m1 = m0+g0, m2 = m1+g1 summed inside PSUM) or a constant block that adds
    the bias: stationary row 0 holds 16*b over a moving ONES block.
  * Weights scaled x16 (alpha-matrices x64) to dodge fp8e4 subnormals;
    descaling folded into the evacuation scale and the STT scalar.
  * No Pool m-adds: m1/m2 are never materialized (PE pairs (m0|g0), (g0|g1)).
  * Per depth: one Act evacuation of ps_y (scale 1/16 -> bf16) + one DVE
    STT g = (ps_B * 1/64) * y -> fp8. Assignment of the evacuations between
    Act and DVE is tuned so both engines carry ~equal load.

Math identical to v1 (see reference): per pair x = lrelu(pv_i + dv_j):
  loop d: y = x@Wd + bd; a = y@att_d + attb_d; x += a*y
  s += relu((m1 + g1 + 2*g2) @ W1a + b1a);  head MLP on s, u_p, u_d.
"""

import numpy as np

import concourse.bass as bass
import concourse.mybir as mybir
import concourse.tile as tile
from concourse.masks import make_identity

F32 = mybir.dt.float32
BF16 = mybir.dt.bfloat16
F8 = mybir.dt.float8e4
ALU = mybir.AluOpType
ACTF = mybir.ActivationFunctionType
DRM = mybir.MatmulPerfMode.DoubleRow

H = 128
NP = 512
ND = 96
NPAIR = NP * ND  # 49152
N_CORES = 8
G = 2            # drugs per group (overridable via knobs["G"])
NG = ND // G
GF = G * NP

SW = 16.0        # W-side fp8 scale
SR = 64.0        # alpha-side fp8 scale

_INPUT_SPECS = {
    "protein_features": ([NP, H], True),
    "drug_features": ([ND, H], True),
    "pu_mask": ([NP], True),
    "du_mask": ([ND], True),
    "d_W": ([H, H], False), "d_b": ([H], False),
    "p_W": ([H, H], False), "p_b": ([H], False),
    "Wv_p": ([H, H], False), "Wv_d": ([H, H], False), "Wv_b": ([H], False),
    "att_W": ([3, H, 1], False), "att_b": ([3, 1], False),
    "Wvs_W": ([3, H, H], False), "Wvs_b": ([3, H], False),
    "Wu_W": ([2 * H, H], False), "Wu_b": ([H], False),
    "W1a_W": ([H, H], False), "W1a_b": ([H], False),
    "W1b_W": ([H, H], False), "W1b_b": ([H], False),
    "W2a_W": ([H, 2 * H], False), "W2a_b": ([2 * H], False),
    "W2b_W": ([2 * H, H], False), "W2b_b": ([H], False),
    "W3_W": ([H, H], False), "W3_b": ([H], False),
    "Wu1a_W": ([2 * H, H], False), "Wu1a_b": ([H], False),
    "Wu1b_W": ([H, H], False), "Wu1b_b": ([H], False),
    "W5_W": ([H, 1], False), "W5_b": ([1], False),
}

_LCNT = [0]


def _lrelu_col(nc, pool, psum_col, bias_col, slope):
    _LCNT[0] += 1
    o = pool.tile([H, 1], F32, tag="lo%d" % _LCNT[0])
    nc.scalar.activation(o, psum_col, ACTF.Prelu, bias=bias_col, alpha=slope)
    return o


def _legalize_multiwaits(nc):
    """Split multi-wait sync_infos (walrus supports one wait per instr)."""
    uid = [0]
    for fn in nc.m.functions:
        for blk in fn.blocks:
            out = []
            changed = False
            for inst in blk.instructions:
                si = inst.sync_info
                waits = list(si.on_wait) if si is not None else []
                if len(waits) > 1:
                    for w in waits[:-1]:
                        uid[0] += 1
                        ev = mybir.InstEventSemaphore(
                            name="I-mwsplit-%d" % uid[0], ins=[], outs=[],
                            engine=inst.engine)
                        ev.sync_info = mybir.SyncInfo(on_wait=[w], on_update=[])
                        out.append(ev)
                    inst.sync_info = mybir.SyncInfo(
                        on_wait=[waits[-1]], on_update=list(si.on_update))
                    changed = True
                out.append(inst)
            if changed:
                blk.instructions = out


def build_bass(knobs=None):
    nc = bass.Bass()
    din = {}
    for name, (shape, _) in _INPUT_SPECS.items():
        din[name] = nc.declare_dram_parameter(name, list(shape), F32, isOutput=False)
    dout = nc.declare_dram_parameter("out", [1, 1], F32, isOutput=True)

    with tile.TileContext(nc) as tc:
        _emit(nc, tc, din, dout, knobs or {})
    _legalize_multiwaits(nc)
    return nc


def _emit(nc, tc, din, dout, knobs):
    import contextlib

    # evac assignment: which engine evacuates ps_y per depth; 'split' puts
    # cols [0:split_at) on Act and the rest on DVE.
    global G, NG, GF
    G = knobs.get("G", 1)
    NG = ND // G
    GF = G * NP
    evac_eng = knobs.get("evac_eng", ("act", "act", "act"))
    alt_mod = knobs.get("alt_mod", 6)
    zs_mod = knobs.get("zs_mod", 8)
    split_at = knobs.get("split_at", 192)
    mega_bufs = knobs.get("mega_bufs", 6)

    ctx = contextlib.ExitStack()
    with ctx:
        const = ctx.enter_context(tc.tile_pool(name="const", bufs=1))
        setup = ctx.enter_context(tc.tile_pool(name="setup", bufs=2))
        spsum_cm = tc.tile_pool(name="spsum", bufs=1, space="PSUM")
        spsum = spsum_cm.__enter__()

        _uid = [0]

        def _tag(p):
            _uid[0] += 1
            return "%s%d" % (p, _uid[0])

        def load(name, src=None, shape=None, q=None):
            src = src if src is not None else din[name]
            if not isinstance(src, bass.AP):
                src = src[:]
            t = const.tile(shape or list(src.shape), F32, tag=_tag("w"))
            (q or nc.sync).dma_start(out=t, in_=src)
            return t

        def load_col(src, n=H, q=None):
            if not isinstance(src, bass.AP):
                src = src[:]
            t = const.tile([n, 1], F32, tag=_tag("b"))
            (q or nc.sync).dma_start(out=t, in_=src.rearrange("(n o) -> n o", o=1))
            return t

        def load_row(src, n=H, q=None):
            if not isinstance(src, bass.AP):
                src = src[:]
            t = const.tile([1, n], F32, tag=_tag("r"))
            (q or nc.sync).dma_start(out=t, in_=src.rearrange("(o n) -> o n", o=1))
            return t

        # ---------------- setup emission order ---------------------------
        # Pool first: identity (transposes need it) before any gpsimd DMA.
        ident = const.tile([H, H], F32)
        make_identity(nc, ident)
        ones_row = const.tile([1, H], F32)
        nc.vector.memset(ones_row, 1.0)
        ones8 = const.tile([H, H], F8)
        nc.vector.memset(ones8, 1.0)

        # One shared HWDGE line: minimize pre-loop DMA count (merge the [3,..]
        # weight stacks into single DMAs) and order strictly by need.
        nat4 = setup.tile([H, 4 * H], F32, tag="nat4")
        nc.sync.dma_start(
            out=nat4[:, :].rearrange("p (c m) -> p c m", c=4),
            in_=din["protein_features"][:].rearrange("(c p) m -> p c m", c=4))
        natd = setup.tile([H, H], F32, tag="natd")
        nc.sync.dma_start(out=natd[0:ND, :], in_=din["drug_features"][:, :])
        p_W = load("p_W")
        d_W = load("d_W")
        p_b = load_col(din["p_b"])
        pu_row = load_row(din["pu_mask"], n=NP)
        d_b = load_col(din["d_b"])
        du_row = load_row(din["du_mask"], n=ND)
        Wv_p = load("Wv_p")
        Wv_d = load("Wv_d")
        nat_pt = [nat4[:, t * H:(t + 1) * H] for t in range(4)]

        # Stationary sources on the gpsimd queue (separate SWDGE line on the
        # idle Pool engine), merged into one DMA per parameter stack.
        gq = nc.gpsimd
        Wvs3 = const.tile([H, 3 * H], F32)
        gq.dma_start(out=Wvs3[:, :].rearrange("p (d m) -> p d m", d=3),
                     in_=din["Wvs_W"][:].rearrange("d p m -> p d m"))
        attW3 = const.tile([H, 3], F32)
        gq.dma_start(out=attW3[:, :].rearrange("p (d o) -> p d o", d=3),
                     in_=din["att_W"][:].rearrange("d p o -> p d o"))
        bcol3 = const.tile([H, 3], F32)
        gq.dma_start(out=bcol3, in_=din["Wvs_b"][:].rearrange("d p -> p d"))
        attb3 = const.tile([1, 3], F32)
        gq.dma_start(out=attb3, in_=din["att_b"][:].rearrange("d o -> o d"))
        Wv_b = load_col(din["Wv_b"], q=gq)
        W1a = load("W1a_W", q=gq)
        W1a_b = load_col(din["W1a_b"], q=gq)
        Wvs = [Wvs3[:, d * H:(d + 1) * H] for d in range(3)]
        attW = [attW3[:, d:d + 1] for d in range(3)]
        bcol = [bcol3[:, d:d + 1] for d in range(3)]
        att_b = [attb3[:, d:d + 1] for d in range(3)]

        # ---------------- transposes: PTt [H,NP], DRt [H,ND] -------------
        PTt = const.tile([H, NP], F32)
        for t in range(4):
            ps = spsum.tile([H, H], F32, tag="tp%d" % (t % 2))
            nc.tensor.transpose(ps, nat_pt[t], ident)
            nc.scalar.activation(PTt[:, t * H:(t + 1) * H], ps, ACTF.Copy)
        DRt = const.tile([H, ND], F32)
        psd = spsum.tile([H, H], F32, tag="tp0")
        nc.tensor.transpose(psd[:, 0:ND], natd[0:ND, :], ident[0:ND, 0:ND])
        nc.scalar.activation(DRt, psd[:, 0:ND], ACTF.Copy)

        # ---------------- stage 1: features ------------------------------
        def feat(WT, Xt, b_col, mask_row, n):
            ps = spsum.tile([H, NP], F32, tag="s1p")
            for c0 in range(0, n, H):
                c1 = min(c0 + H, n)
                nc.tensor.matmul(ps[:, c0:c1], WT, Xt[:, c0:c1],
                                 start=True, stop=True)
            l = setup.tile([H, NP], F32, tag="s1l")
            nc.scalar.activation(l[:, 0:n], ps[:, 0:n], ACTF.Prelu,
                                 bias=b_col, alpha=0.1)
            pm = spsum.tile([H, NP], F32, tag="s1m")
            nc.tensor.matmul(pm[:, 0:n], ones_row, mask_row, start=True, stop=True)
            f = setup.tile([H, NP], F32, tag="s1f")
            nc.vector.scalar_tensor_tensor(
                f[:, 0:n], l[:, 0:n], 1.0, pm[:, 0:n], ALU.mult, ALU.mult)
            return f

        prot = feat(p_W, PTt, p_b, pu_row, NP)      # [128, 512] f32
        drug = feat(d_W, DRt, d_b, du_row, ND)      # [128, 96]

        u_p = const.tile([H, 1], F32)
        nc.vector.tensor_reduce(u_p, prot[:, 0:NP], mybir.AxisListType.X, ALU.add)
        u_d = const.tile([H, 1], F32)
        nc.vector.tensor_reduce(u_d, drug[:, 0:ND], mybir.AxisListType.X, ALU.add)

        ps_pv = spsum.tile([H, NP], F32, tag="s1p")
        nc.tensor.matmul(ps_pv, Wv_p, prot[:, 0:NP], start=True, stop=True)
        pv = const.tile([H, NP], F32)
        nc.scalar.activation(pv, ps_pv, ACTF.Copy)

        ps_dv = spsum.tile([H, ND], F32, tag="s1m")
        nc.tensor.matmul(ps_dv, Wv_d, drug[:, 0:ND], start=True, stop=True)
        dvf = const.tile([H, ND], F32)
        nc.scalar.activation(dvf, ps_dv, ACTF.Identity, bias=Wv_b)

        # ---------------- fp8 stationaries [128, 256] --------------------
        # statY_d  = [b-row-block | 16*Wvs_d]     pairs rhs (ONES | m0)
        # statYz_d = [0 | 16*Wvs_d]               pairs rhs (m0 | g0) etc.
        # statB_d  = [c-row-block | SRd*R_d]      pairs rhs (ONES | m0)
        # statBz_d = [0 | SRd*R_d]
        # statV    = [16*W1a | 16*W1a]
        statY, statYz, statB, statBz = [], [], [], []
        c_col, b_ev = [], []
        for d in range(3):
            srd = SR * (2.0 if d == 2 else 1.0)
            # A_d = Wvs_d @ att_d  via transpose then matmul (as in v1)
            psT = spsum.tile([H, H], F32, tag="tps")
            nc.tensor.transpose(psT, Wvs[d], ident)
            WvsT = setup.tile([H, H], F32, tag="wvsT")
            nc.vector.tensor_copy(WvsT, psT)
            psA = spsum.tile([H, 1], F32, tag="smu")
            nc.tensor.matmul(psA, WvsT, attW[d], start=True, stop=True)
            A_col = setup.tile([H, 1], F32, tag="acol")
            nc.vector.tensor_scalar(A_col, psA, srd, None, ALU.mult)
            R8 = setup.tile([H, H], F8, tag="r8")
            nc.vector.tensor_scalar(R8, ones8, A_col, None, ALU.mult)

            # c_d = b_d.att_d + att_b_d; as [128,1] col times srd (for the
            # STT scalar-add), and b_d/SR col (for the evac bias).
            psc = spsum.tile([1, 1], F32, tag="psc")
            nc.tensor.matmul(psc, bcol[d], attW[d], start=True, stop=True)
            c1 = setup.tile([1, 1], F32, tag="c1")
            nc.vector.tensor_scalar(c1, psc, att_b[d], srd, ALU.add, ALU.mult)
            pscb = spsum.tile([H, 1], F32, tag="smu")
            nc.tensor.matmul(pscb, ones_row, c1, start=True, stop=True)
            cc = const.tile([H, 1], F32, tag=_tag("cc"))
            nc.scalar.activation(cc, pscb, ACTF.Copy)
            c_col.append(cc)
            bb = const.tile([H, 1], F32, tag=_tag("bb"))
            nc.vector.tensor_scalar(bb, bcol[d], 1.0 / SR, None, ALU.mult)
            b_ev.append(bb)

            sy = const.tile([H, 2 * H], F8, tag=_tag("st"))
            nc.gpsimd.memset(sy[:, 0:H], 0.0)
            nc.vector.tensor_scalar(sy[:, H:2 * H], Wvs[d], SW, None, ALU.mult)
            statY.append(sy)
            statYz.append(sy)

            sb = const.tile([H, 2 * H], F8, tag=_tag("st"))
            nc.gpsimd.memset(sb[:, 0:H], 0.0)
            nc.vector.tensor_copy(sb[:, H:2 * H], R8)
            statB.append(sb)
            statBz.append(sb)

        statV = const.tile([H, 2 * H], F8)
        nc.vector.tensor_scalar(statV[:, 0:H], W1a, SW, None, ALU.mult)
        nc.vector.tensor_scalar(statV[:, H:2 * H], W1a, SW, None, ALU.mult)

        # for d2 second block: statY2p = [16W2 | 16W2], statB2p = [R2' | R2']
        statY2p = const.tile([H, 2 * H], F8)
        nc.vector.tensor_scalar(statY2p[:, 0:H], Wvs[2], SW, None, ALU.mult)
        nc.vector.tensor_scalar(statY2p[:, H:2 * H], Wvs[2], SW, None, ALU.mult)
        statB2p = const.tile([H, 2 * H], F8)
        nc.vector.tensor_copy(statB2p[:, 0:H], statBz[2][:, H:2 * H])
        nc.vector.tensor_copy(statB2p[:, H:2 * H], statBz[2][:, H:2 * H])

        b1a16 = const.tile([H, 1], F32)
        nc.vector.tensor_scalar(b1a16, W1a_b, SW, None, ALU.mult)
        zeros_bf = const.tile([H, NP], BF16)
        nc.gpsimd.memset(zeros_bf, 0.0)

        sacc = const.tile([H, NG], F32)

        spsum_cm.__exit__(None, None, None)

        # ---------------- pairwise main loop: 48 groups of 2 drugs -------
        # mega slot layout (fp8, per group): [ONES | m0 | g0 | g1 | g2],
        # each [128, GF]. DR rhs for tile t of the pair: view two adjacent
        # slots as [128, 2, GF] and take [:, :, t*NP:(t+1)*NP].
        SLOT = GF

        def pair_ap(mega, s, t):
            v = mega[:, s * SLOT:(s + 2) * SLOT]
            v = v.rearrange("p (a b) -> p a b", a=2)
            return v[:, :, t * NP:(t + 1) * NP]

        # PSUM plan (8 banks of [128,512]f32), all rotations benign:
        #   psY: y [128,1024] bufs=2 (4 banks)  y0/y1/y2 rotation
        #   psB: B [128,512] per-tile bufs=2 (2 banks)
        #   psS: s4 [128,1024] bufs=1 (2 banks)
        with tc.tile_pool(name="mega", bufs=mega_bufs) as megap, \
             tc.tile_pool(name="ysb", bufs=knobs.get("ysb_bufs", 4)) as ysbp, \
             tc.tile_pool(name="psY", bufs=knobs.get("psy_bufs", 3), space="PSUM") as psY, \
             tc.tile_pool(name="psBt", bufs=knobs.get("psb_bufs", 3), space="PSUM") as psBt, \
             tc.tile_pool(name="psS", bufs=knobs.get("pss_bufs", 2), space="PSUM") as psS:

            def st_m0(st):
                g = st["g"]
                mega = megap.tile([H, 5 * SLOT], F8, tag="mega")
                if g < mega_bufs:
                    nc.gpsimd.memset(mega[:, 0:SLOT], 1.0)  # ONES slot, once/buf
                st["mega"] = mega
                for t in range(G):
                    j = g * G + t
                    nc.scalar.activation(
                        mega[:, SLOT + t * NP: SLOT + (t + 1) * NP], pv,
                        ACTF.Prelu, bias=dvf[:, j:j + 1], alpha=0.1)

            def st_pe0(st):
                mega = st["mega"]
                py = psY.tile([H, GF], F32, tag="y")
                pbs = []
                for t in range(G):
                    rhs = pair_ap(mega, 0, t)   # (ONES | m0)
                    nc.tensor.matmul(py[:, t * NP:(t + 1) * NP],
                                     statY[0][:, :].rearrange("p (a b) -> p a b", a=2),
                                     rhs, start=True, stop=True, perf_mode=DRM)
                    pb = psBt.tile([H, NP], F32, tag="b")
                    nc.tensor.matmul(pb,
                                     statB[0][:, :].rearrange("p (a b) -> p a b", a=2),
                                     rhs, start=True, stop=True, perf_mode=DRM)
                    pbs.append(pb)
                st["py"], st["pb"] = py, pbs

            def _evac(st, d):
                py = st["py"]
                y = ysbp.tile([H, GF], BF16, tag="y%d" % d)
                eng = evac_eng[d]
                if eng == "alt":
                    eng = "act" if (st["g"] % alt_mod) == 0 else "dve"
                if eng == "act":
                    nc.scalar.activation(y, py, ACTF.Identity, bias=b_ev[d],
                                         scale=1.0 / (SW * SR))
                else:
                    nc.vector.tensor_scalar(y, py, 1.0 / (SW * SR), b_ev[d],
                                            ALU.mult, ALU.add)
                st["y"] = y

            def _stt(st, d, slot):
                # g_d = (ps_B * 1/SR) * y -> fp8 into mega slot (per tile)
                mega = st["mega"]
                for t in range(G):
                    o = slice(slot * SLOT + t * NP, slot * SLOT + (t + 1) * NP)
                    nc.vector.scalar_tensor_tensor(
                        mega[:, o], st["pb"][t], c_col[d],
                        st["y"][:, t * NP:(t + 1) * NP], ALU.add, ALU.mult)

            def st_ev0(st):
                _evac(st, 0)

            def st_g0(st):
                _stt(st, 0, 2)

            def st_pe1(st):
                mega = st["mega"]
                py = psY.tile([H, GF], F32, tag="y")
                pbs = []
                for t in range(G):
                    r_om = pair_ap(mega, 0, t)   # (ONES | m0)
                    r_mg = pair_ap(mega, 1, t)   # (m0 | g0)
                    o = slice(t * NP, (t + 1) * NP)
                    nc.tensor.matmul(py[:, o], statY[1][:, :].rearrange("p (a b) -> p a b", a=2),
                                     r_om, start=True, stop=False, perf_mode=DRM)
                    nc.tensor.matmul(py[:, o], statYz[1][:, :].rearrange("p (a b) -> p a b", a=2),
                                     r_mg, start=False, stop=True, perf_mode=DRM)
                    pb = psBt.tile([H, NP], F32, tag="b")
                    nc.tensor.matmul(pb, statB[1][:, :].rearrange("p (a b) -> p a b", a=2),
                                     r_om, start=True, stop=False, perf_mode=DRM)
                    nc.tensor.matmul(pb, statBz[1][:, :].rearrange("p (a b) -> p a b", a=2),
                                     r_mg, start=False, stop=True, perf_mode=DRM)
                    pbs.append(pb)
                st["py"], st["pb"] = py, pbs

            def st_ev1(st):
                _evac(st, 1)

            def st_g1(st):
                _stt(st, 1, 3)

            def st_pe2(st):
                mega = st["mega"]
                py = psY.tile([H, GF], F32, tag="y")
                pbs = []
                for t in range(G):
                    r_om = pair_ap(mega, 0, t)   # (ONES | m0)
                    r_gg = pair_ap(mega, 2, t)   # (g0 | g1)
                    o = slice(t * NP, (t + 1) * NP)
                    nc.tensor.matmul(py[:, o], statY[2][:, :].rearrange("p (a b) -> p a b", a=2),
                                     r_om, start=True, stop=False, perf_mode=DRM)
                    nc.tensor.matmul(py[:, o], statY2p[:, :].rearrange("p (a b) -> p a b", a=2),
                                     r_gg, start=False, stop=True, perf_mode=DRM)
                    pb = psBt.tile([H, NP], F32, tag="b")
                    nc.tensor.matmul(pb, statB[2][:, :].rearrange("p (a b) -> p a b", a=2),
                                     r_om, start=True, stop=False, perf_mode=DRM)
                    nc.tensor.matmul(pb, statB2p[:, :].rearrange("p (a b) -> p a b", a=2),
                                     r_gg, start=False, stop=True, perf_mode=DRM)
                    pbs.append(pb)
                st["py"], st["pb"] = py, pbs

            def st_ev2(st):
                _evac(st, 2)

            def st_g2(st):
                _stt(st, 2, 4)

            def st_pe4(st):
                mega = st["mega"]
                p4 = psS.tile([H, GF], F32, tag="s4")
                for t in range(G):
                    r_mg = pair_ap(mega, 1, t)   # (m0 | g0)
                    r_gg2 = pair_ap(mega, 3, t)  # (g1 | g2)
                    o = slice(t * NP, (t + 1) * NP)
                    nc.tensor.matmul(p4[:, o], statV[:, :].rearrange("p (a b) -> p a b", a=2),
                                     r_mg, start=True, stop=False, perf_mode=DRM)
                    nc.tensor.matmul(p4[:, o], statV[:, :].rearrange("p (a b) -> p a b", a=2),
                                     r_gg2, start=False, stop=True, perf_mode=DRM)
                st["p4"] = p4

            def st_zs(st):
                g = st["g"]
                if (g % zs_mod) == 0:
                    zso = ysbp.tile([H, GF], F8, tag="zso")
                    nc.scalar.activation(zso, st["p4"], ACTF.Relu, bias=b1a16,
                                         scale=1.0,
                                         accum_out=sacc[:, g:g + 1])
                else:
                    zso = ysbp.tile([H, GF], BF16, tag="zsv")
                    nc.vector.scalar_tensor_tensor(
                        zso, st["p4"], b1a16, zeros_bf[:, 0:GF],
                        ALU.add, ALU.max, accum_out=sacc[:, g:g + 1])

            stages = [st_m0, st_pe0, st_ev0, st_g0, st_pe1, st_ev1, st_g1,
                      st_pe2, st_ev2, st_g2, st_pe4, st_zs]
            # slot offset of each stage within a group's schedule; groups are
            # spaced SPACING slots apart.
            OFS = knobs.get("ofs", [0, 0, 1, 1, 2, 2, 3, 3, 4, 4, 5, 6])
            SPACING = knobs.get("spacing", 1)
            NST = len(stages)
            live = {}
            total_slots = (NG - 1) * SPACING + OFS[-1] + 1
            # within a slot: oldest group (largest OFS) first; within a group
            # (equal OFS), ascending stage order.
            order = sorted(range(NST), key=lambda s: (-OFS[s], s))
            for slot in range(total_slots):
                for s in order:
                    rem = slot - OFS[s]
                    if rem < 0 or rem % SPACING != 0:
                        continue
                    g = rem // SPACING
                    if g < 0 or g >= NG:
                        continue
                    if s == 0:
                        live[g] = {"g": g}
                    stages[s](live[g])
                    if s == NST - 1:
                        del live[g]

        # ---------------- head (tiny, f32) -------------------------------
        W1b = load("W1b_W", q=gq); W3 = load("W3_W", q=gq)
        Wu1b = load("Wu1b_W"); W5 = load("W5_W")
        W2a_lo = load(None, src=din["W2a_W"][:, 0:H])
        W2a_hi = load(None, src=din["W2a_W"][:, H:2 * H])
        W2b_lo = load(None, src=din["W2b_W"][0:H, :])
        W2b_hi = load(None, src=din["W2b_W"][H:2 * H, :])
        Wu1a_lo = load(None, src=din["Wu1a_W"][0:H, :])
        Wu1a_hi = load(None, src=din["Wu1a_W"][H:2 * H, :])
        Wu_lo = load(None, src=din["Wu_W"][0:H, :])
        Wu_hi = load(None, src=din["Wu_W"][H:2 * H, :])
        Wu_b = load_col(din["Wu_b"]); W1b_b = load_col(din["W1b_b"])
        W2a_b_lo = load_col(din["W2a_b"][0:H]); W2a_b_hi = load_col(din["W2a_b"][H:2 * H])
        W2b_b = load_col(din["W2b_b"]); W3_b = load_col(din["W3_b"])
        Wu1a_b = load_col(din["Wu1a_b"]); Wu1b_b = load_col(din["Wu1b_b"])
        W5_b = load_col(din["W5_b"], n=1)

        with tc.tile_pool(name="head", bufs=1) as hp, \
             tc.tile_pool(name="hpsum", bufs=1, space="PSUM") as hps:
            ps_mu = hps.tile([H, 1], F32, tag="h1")
            nc.tensor.matmul(ps_mu, Wu_lo, u_d, start=True, stop=False)
            nc.tensor.matmul(ps_mu, Wu_hi, u_p, start=False, stop=True)
            m_u = _lrelu_col(nc, hp, ps_mu, Wu_b, 0.01)

            s_raw = hp.tile([H, 1], F32)
            nc.vector.tensor_reduce(s_raw, sacc, mybir.AxisListType.X, ALU.add)
            s_col = hp.tile([H, 1], F32)
            nc.vector.tensor_scalar(s_col, s_raw, 1.0 / SW, None, ALU.mult)

            ps_g1 = hps.tile([H, 1], F32, tag="h1")
            nc.tensor.matmul(ps_g1, W1b, s_col, start=True, stop=True)
            w1bbN = hp.tile([H, 1], F32)
            nc.vector.tensor_scalar(w1bbN, W1b_b, float(NPAIR), None, ALU.mult)
            g1 = hp.tile([H, 1], F32)
            nc.vector.tensor_scalar(g1, ps_g1, w1bbN, None, ALU.add)

            ps_lo = hps.tile([H, 1], F32, tag="h2")
            nc.tensor.matmul(ps_lo, W2a_lo, g1, start=True, stop=True)
            t_lo = _lrelu_col(nc, hp, ps_lo, W2a_b_lo, 0.1)
            ps_hi = hps.tile([H, 1], F32, tag="h3")
            nc.tensor.matmul(ps_hi, W2a_hi, g1, start=True, stop=True)
            t_hi = _lrelu_col(nc, hp, ps_hi, W2a_b_hi, 0.1)

            ps_g2 = hps.tile([H, 1], F32, tag="h4")
            nc.tensor.matmul(ps_g2, W2b_lo, t_lo, start=True, stop=False)
            nc.tensor.matmul(ps_g2, W2b_hi, t_hi, start=False, stop=True)
            g2 = hp.tile([H, 1], F32)
            nc.vector.tensor_scalar(g2, ps_g2, W2b_b, None, ALU.add)

            ps_g3 = hps.tile([H, 1], F32, tag="h5")
            nc.tensor.matmul(ps_g3, W3, g2, start=True, stop=True)
            g3 = _lrelu_col(nc, hp, ps_g3, W3_b, 0.1)

            ps_u = hps.tile([H, 1], F32, tag="h6")
            nc.tensor.matmul(ps_u, Wu1a_lo, m_u, start=True, stop=False)
            nc.tensor.matmul(ps_u, Wu1a_hi, g3, start=False, stop=True)
            h1 = _lrelu_col(nc, hp, ps_u, Wu1a_b, 0.1)

            ps_mu2 = hps.tile([H, 1], F32, tag="h7")
            nc.tensor.matmul(ps_mu2, Wu1b, h1, start=True, stop=True)
            mu = hp.tile([H, 1], F32)
            nc.vector.tensor_scalar(mu, ps_mu2, Wu1b_b, None, ALU.add)

            ps_o = hps.tile([1, 1], F32, tag="h8")
            nc.tensor.matmul(ps_o, W5, mu, start=True, stop=True)
            res = hp.tile([1, 1], F32)
            nc.vector.tensor_scalar(res, ps_o, W5_b, None, ALU.add)
            nc.sync.dma_start(out=dout[:, :], in_=res)


_CACHE = {}


def _get_nc():
    if "nc" not in _CACHE:
        _CACHE["nc"] = build_bass()
    return _CACHE["nc"]


def kernel(**inputs):
    from concourse.bass_utils import run_bass_kernel_spmd

    nc = _get_nc()
    per_core = {"protein_features", "drug_features", "pu_mask", "du_mask"}
    in_maps = []
    for b in range(N_CORES):
        m = {}
        for name in _INPUT_SPECS:
            arr = np.asarray(inputs[name], dtype=np.float32)
            m[name] = np.ascontiguousarray(arr[b]) if name in per_core else arr
        in_maps.append(m)
    res = run_bass_kernel_spmd(nc, in_maps, list(range(N_CORES)))
    out = np.stack([res.results[i]["out"].reshape(1) for i in range(N_CORES)])
    return out.astype(np.float32)


if __name__ == "__main__":
    nc = build_bass()
    print("build ok")


# revision 31
# speedup vs baseline: 1.5310x; 1.0010x over previous
"""Trainium2 Bass kernel for nn_DTIModel (DTI pairwise message passing), v2.

Sharding: data-parallel over batch B=8 across 8 NeuronCores (one batch element
per core, weights replicated). The [Np*Nd, H] pairwise tensor never touches
DRAM: produced, attended (D=3) and reduced on-chip.

v2 design (vs the bf16 v1 baseline):
  * Tiles processed in PAIRS (G=2 drugs -> [128, 1024] elementwise ops) to
    amortize fixed per-op costs; the 96 drug-tiles are fully independent.
  * All pairwise matmuls are fp8e4 DoubleRow (0.5 cyc/row): the second
    contraction block of each DR pair carries either real data (virtual
    m1 = m0+g0, m2 = m1+g1 summed inside PSUM) or a constant block that adds
    the bias: stationary row 0 holds 16*b over a moving ONES block.
  * Weights scaled x16 (alpha-matrices x64) to dodge fp8e4 subnormals;
    descaling folded into the evacuation scale and the STT scalar.
  * No Pool m-adds: m1/m2 are never materialized (PE pairs (m0|g0), (g0|g1)).
  * Per depth: one Act evacuation of ps_y (scale 1/16 -> bf16) + one DVE
    STT g = (ps_B * 1/64) * y -> fp8. Assignment of the evacuations between
    Act and DVE is tuned so both engines carry ~equal load.

Math identical to v1 (see reference): per pair x = lrelu(pv_i + dv_j):
  loop d: y = x@Wd + bd; a = y@att_d + attb_d; x += a*y
  s += relu((m1 + g1 + 2*g2) @ W1a + b1a);  head MLP on s, u_p, u_d.
"""

import numpy as np

import concourse.bass as bass
import concourse.mybir as mybir
import concourse.tile as tile
from concourse.masks import make_identity

F32 = mybir.dt.float32
BF16 = mybir.dt.bfloat16
F8 = mybir.dt.float8e4
ALU = mybir.AluOpType
ACTF = mybir.ActivationFunctionType
DRM = mybir.MatmulPerfMode.DoubleRow

H = 128
NP = 512
ND = 96
NPAIR = NP * ND  # 49152
N_CORES = 8
G = 2            # drugs per group (overridable via knobs["G"])
NG = ND // G
GF = G * NP

SW = 16.0        # W-side fp8 scale
SR = 64.0        # alpha-side fp8 scale

_INPUT_SPECS = {
    "protein_features": ([NP, H], True),
    "drug_features": ([ND, H], True),
    "pu_mask": ([NP], True),
    "du_mask": ([ND], True),
    "d_W": ([H, H], False), "d_b": ([H], False),
    "p_W": ([H, H], False), "p_b": ([H], False),
    "Wv_p": ([H, H], False), "Wv_d": ([H, H], False), "Wv_b": ([H], False),
    "att_W": ([3, H, 1], False), "att_b": ([3, 1], False),
    "Wvs_W": ([3, H, H], False), "Wvs_b": ([3, H], False),
    "Wu_W": ([2 * H, H], False), "Wu_b": ([H], False),
    "W1a_W": ([H, H], False), "W1a_b": ([H], False),
    "W1b_W": ([H, H], False), "W1b_b": ([H], False),
    "W2a_W": ([H, 2 * H], False), "W2a_b": ([2 * H], False),
    "W2b_W": ([2 * H, H], False), "W2b_b": ([H], False),
    "W3_W": ([H, H], False), "W3_b": ([H], False),
    "Wu1a_W": ([2 * H, H], False), "Wu1a_b": ([H], False),
    "Wu1b_W": ([H, H], False), "Wu1b_b": ([H], False),
    "W5_W": ([H, 1], False), "W5_b": ([1], False),
}

_LCNT = [0]


def _lrelu_col(nc, pool, psum_col, bias_col, slope):
    _LCNT[0] += 1
    o = pool.tile([H, 1], F32, tag="lo%d" % _LCNT[0])
    nc.scalar.activation(o, psum_col, ACTF.Prelu, bias=bias_col, alpha=slope)
    return o


def _legalize_multiwaits(nc):
    """Split multi-wait sync_infos (walrus supports one wait per instr)."""
    uid = [0]
    for fn in nc.m.functions:
        for blk in fn.blocks:
            out = []
            changed = False
            for inst in blk.instructions:
                si = inst.sync_info
                waits = list(si.on_wait) if si is not None else []
                if len(waits) > 1:
                    for w in waits[:-1]:
                        uid[0] += 1
                        ev = mybir.InstEventSemaphore(
                            name="I-mwsplit-%d" % uid[0], ins=[], outs=[],
                            engine=inst.engine)
                        ev.sync_info = mybir.SyncInfo(on_wait=[w], on_update=[])
                        out.append(ev)
                    inst.sync_info = mybir.SyncInfo(
                        on_wait=[waits[-1]], on_update=list(si.on_update))
                    changed = True
                out.append(inst)
            if changed:
                blk.instructions = out


def build_bass(knobs=None):
    nc = bass.Bass()
    din = {}
    for name, (shape, _) in _INPUT_SPECS.items():
        din[name] = nc.declare_dram_parameter(name, list(shape), F32, isOutput=False)
    dout = nc.declare_dram_parameter("out", [1, 1], F32, isOutput=True)

    with tile.TileContext(nc) as tc:
        _emit(nc, tc, din, dout, knobs or {})
    _legalize_multiwaits(nc)
    return nc


def _emit(nc, tc, din, dout, knobs):
    import contextlib

    # evac assignment: which engine evacuates ps_y per depth; 'split' puts
    # cols [0:split_at) on Act and the rest on DVE.
    global G, NG, GF
    G = knobs.get("G", 1)
    NG = ND // G
    GF = G * NP
    evac_eng = knobs.get("evac_eng", ("act", "act", "act"))
    alt_mod = knobs.get("alt_mod", 6)
    zs_mod = knobs.get("zs_mod", 8)
    split_at = knobs.get("split_at", 192)
    mega_bufs = knobs.get("mega_bufs", 6)

    ctx = contextlib.ExitStack()
    with ctx:
        const = ctx.enter_context(tc.tile_pool(name="const", bufs=1))
        setup = ctx.enter_context(tc.tile_pool(name="setup", bufs=2))
        spsum_cm = tc.tile_pool(name="spsum", bufs=1, space="PSUM")
        spsum = spsum_cm.__enter__()

        _uid = [0]

        def _tag(p):
            _uid[0] += 1
            return "%s%d" % (p, _uid[0])

        def load(name, src=None, shape=None, q=None):
            src = src if src is not None else din[name]
            if not isinstance(src, bass.AP):
                src = src[:]
            t = const.tile(shape or list(src.shape), F32, tag=_tag("w"))
            (q or nc.sync).dma_start(out=t, in_=src)
            return t

        def load_col(src, n=H, q=None):
            if not isinstance(src, bass.AP):
                src = src[:]
            t = const.tile([n, 1], F32, tag=_tag("b"))
            (q or nc.sync).dma_start(out=t, in_=src.rearrange("(n o) -> n o", o=1))
            return t

        def load_row(src, n=H, q=None):
            if not isinstance(src, bass.AP):
                src = src[:]
            t = const.tile([1, n], F32, tag=_tag("r"))
            (q or nc.sync).dma_start(out=t, in_=src.rearrange("(o n) -> o n", o=1))
            return t

        # ---------------- setup emission order ---------------------------
        # Pool first: identity (transposes need it) before any gpsimd DMA.
        ident = const.tile([H, H], F32)
        make_identity(nc, ident)
        ones_row = const.tile([1, H], F32)
        nc.vector.memset(ones_row, 1.0)
        ones8 = const.tile([H, H], F8)
        nc.vector.memset(ones8, 1.0)

        # One shared HWDGE line: minimize pre-loop DMA count (merge the [3,..]
        # weight stacks into single DMAs) and order strictly by need.
        nat4 = setup.tile([H, 4 * H], F32, tag="nat4")
        nc.sync.dma_start(
            out=nat4[:, :].rearrange("p (c m) -> p c m", c=4),
            in_=din["protein_features"][:].rearrange("(c p) m -> p c m", c=4))
        natd = setup.tile([H, H], F32, tag="natd")
        nc.sync.dma_start(out=natd[0:ND, :], in_=din["drug_features"][:, :])
        p_W = load("p_W")
        d_W = load("d_W")
        p_b = load_col(din["p_b"])
        pu_row = load_row(din["pu_mask"], n=NP)
        d_b = load_col(din["d_b"])
        du_row = load_row(din["du_mask"], n=ND)
        Wv_p = load("Wv_p")
        Wv_d = load("Wv_d")
        nat_pt = [nat4[:, t * H:(t + 1) * H] for t in range(4)]

        # Stationary sources on the gpsimd queue (separate SWDGE line on the
        # idle Pool engine), merged into one DMA per parameter stack.
        gq = nc.gpsimd
        Wvs3 = const.tile([H, 3 * H], F32)
        gq.dma_start(out=Wvs3[:, :].rearrange("p (d m) -> p d m", d=3),
                     in_=din["Wvs_W"][:].rearrange("d p m -> p d m"))
        attW3 = const.tile([H, 3], F32)
        gq.dma_start(out=attW3[:, :].rearrange("p (d o) -> p d o", d=3),
                     in_=din["att_W"][:].rearrange("d p o -> p d o"))
        bcol3 = const.tile([H, 3], F32)
        gq.dma_start(out=bcol3, in_=din["Wvs_b"][:].rearrange("d p -> p d"))
        attb3 = const.tile([1, 3], F32)
        gq.dma_start(out=attb3, in_=din["att_b"][:].rearrange("d o -> o d"))
        Wv_b = load_col(din["Wv_b"], q=gq)
        W1a = load("W1a_W", q=gq)
        W1a_b = load_col(din["W1a_b"], q=gq)
        Wvs = [Wvs3[:, d * H:(d + 1) * H] for d in range(3)]
        attW = [attW3[:, d:d + 1] for d in range(3)]
        bcol = [bcol3[:, d:d + 1] for d in range(3)]
        att_b = [attb3[:, d:d + 1] for d in range(3)]

        # ---------------- transposes: PTt [H,NP], DRt [H,ND] -------------
        PTt = const.tile([H, NP], F32)
        for t in range(4):
            ps = spsum.tile([H, H], F32, tag="tp%d" % (t % 2))
            nc.tensor.transpose(ps, nat_pt[t], ident)
            nc.scalar.activation(PTt[:, t * H:(t + 1) * H], ps, ACTF.Copy)
        DRt = const.tile([H, ND], F32)
        psd = spsum.tile([H, H], F32, tag="tp0")
        nc.tensor.transpose(psd[:, 0:ND], natd[0:ND, :], ident[0:ND, 0:ND])
        nc.scalar.activation(DRt, psd[:, 0:ND], ACTF.Copy)

        # ---------------- stage 1: features ------------------------------
        def feat(WT, Xt, b_col, mask_row, n):
            ps = spsum.tile([H, NP], F32, tag="s1p")
            for c0 in range(0, n, H):
                c1 = min(c0 + H, n)
                nc.tensor.matmul(ps[:, c0:c1], WT, Xt[:, c0:c1],
                                 start=True, stop=True)
            l = setup.tile([H, NP], F32, tag="s1l")
            nc.scalar.activation(l[:, 0:n], ps[:, 0:n], ACTF.Prelu,
                                 bias=b_col, alpha=0.1)
            pm = spsum.tile([H, NP], F32, tag="s1m")
            nc.tensor.matmul(pm[:, 0:n], ones_row, mask_row, start=True, stop=True)
            f = setup.tile([H, NP], F32, tag="s1f")
            nc.vector.scalar_tensor_tensor(
                f[:, 0:n], l[:, 0:n], 1.0, pm[:, 0:n], ALU.mult, ALU.mult)
            return f

        prot = feat(p_W, PTt, p_b, pu_row, NP)      # [128, 512] f32
        drug = feat(d_W, DRt, d_b, du_row, ND)      # [128, 96]

        u_p = const.tile([H, 1], F32)
        nc.vector.tensor_reduce(u_p, prot[:, 0:NP], mybir.AxisListType.X, ALU.add)
        u_d = const.tile([H, 1], F32)
        nc.vector.tensor_reduce(u_d, drug[:, 0:ND], mybir.AxisListType.X, ALU.add)

        ps_pv = spsum.tile([H, NP], F32, tag="s1p")
        nc.tensor.matmul(ps_pv, Wv_p, prot[:, 0:NP], start=True, stop=True)
        pv = const.tile([H, NP], F32)
        nc.scalar.activation(pv, ps_pv, ACTF.Copy)

        ps_dv = spsum.tile([H, ND], F32, tag="s1m")
        nc.tensor.matmul(ps_dv, Wv_d, drug[:, 0:ND], start=True, stop=True)
        dvf = const.tile([H, ND], F32)
        nc.scalar.activation(dvf, ps_dv, ACTF.Identity, bias=Wv_b)

        # ---------------- fp8 stationaries [128, 256] --------------------
        # statY_d  = [b-row-block | 16*Wvs_d]     pairs rhs (ONES | m0)
        # statYz_d = [0 | 16*Wvs_d]               pairs rhs (m0 | g0) etc.
        # statB_d  = [c-row-block | SRd*R_d]      pairs rhs (ONES | m0)
        # statBz_d = [0 | SRd*R_d]
        # statV    = [16*W1a | 16*W1a]
        statY, statYz, statB, statBz = [], [], [], []
        c_col, b_ev = [], []
        for d in range(3):
            srd = SR * (2.0 if d == 2 else 1.0)
            # A_d = Wvs_d @ att_d  via transpose then matmul (as in v1)
            psT = spsum.tile([H, H], F32, tag="tps")
            nc.tensor.transpose(psT, Wvs[d], ident)
            WvsT = setup.tile([H, H], F32, tag="wvsT")
            nc.vector.tensor_copy(WvsT, psT)
            psA = spsum.tile([H, 1], F32, tag="smu")
            nc.tensor.matmul(psA, WvsT, attW[d], start=True, stop=True)
            A_col = setup.tile([H, 1], F32, tag="acol")
            nc.vector.tensor_scalar(A_col, psA, srd, None, ALU.mult)
            R8 = setup.tile([H, H], F8, tag="r8")
            nc.vector.tensor_scalar(R8, ones8, A_col, None, ALU.mult)

            # c_d = b_d.att_d + att_b_d; as [128,1] col times srd (for the
            # STT scalar-add), and b_d/SR col (for the evac bias).
            psc = spsum.tile([1, 1], F32, tag="psc")
            nc.tensor.matmul(psc, bcol[d], attW[d], start=True, stop=True)
            c1 = setup.tile([1, 1], F32, tag="c1")
            nc.vector.tensor_scalar(c1, psc, att_b[d], srd, ALU.add, ALU.mult)
            pscb = spsum.tile([H, 1], F32, tag="smu")
            nc.tensor.matmul(pscb, ones_row, c1, start=True, stop=True)
            cc = const.tile([H, 1], F32, tag=_tag("cc"))
            nc.scalar.activation(cc, pscb, ACTF.Copy)
            c_col.append(cc)
            bb = const.tile([H, 1], F32, tag=_tag("bb"))
            nc.vector.tensor_scalar(bb, bcol[d], 1.0 / SR, None, ALU.mult)
            b_ev.append(bb)

            sy = const.tile([H, 2 * H], F8, tag=_tag("st"))
            nc.gpsimd.memset(sy[:, 0:H], 0.0)
            nc.vector.tensor_scalar(sy[:, H:2 * H], Wvs[d], SW, None, ALU.mult)
            statY.append(sy)
            statYz.append(sy)

            sb = const.tile([H, 2 * H], F8, tag=_tag("st"))
            nc.gpsimd.memset(sb[:, 0:H], 0.0)
            nc.vector.tensor_copy(sb[:, H:2 * H], R8)
            statB.append(sb)
            statBz.append(sb)

        statV = const.tile([H, 2 * H], F8)
        nc.vector.tensor_scalar(statV[:, 0:H], W1a, SW, None, ALU.mult)
        nc.vector.tensor_scalar(statV[:, H:2 * H], W1a, SW, None, ALU.mult)

        # for d2 second block: statY2p = [16W2 | 16W2], statB2p = [R2' | R2']
        statY2p = const.tile([H, 2 * H], F8)
        nc.vector.tensor_scalar(statY2p[:, 0:H], Wvs[2], SW, None, ALU.mult)
        nc.vector.tensor_scalar(statY2p[:, H:2 * H], Wvs[2], SW, None, ALU.mult)
        statB2p = const.tile([H, 2 * H], F8)
        nc.vector.tensor_copy(statB2p[:, 0:H], statBz[2][:, H:2 * H])
        nc.vector.tensor_copy(statB2p[:, H:2 * H], statBz[2][:, H:2 * H])

        b1a16 = const.tile([H, 1], F32)
        nc.vector.tensor_scalar(b1a16, W1a_b, SW, None, ALU.mult)
        zeros_bf = const.tile([H, NP], BF16)
        nc.gpsimd.memset(zeros_bf, 0.0)

        sacc = const.tile([H, NG], F32)

        spsum_cm.__exit__(None, None, None)

        # ---------------- pairwise main loop: 48 groups of 2 drugs -------
        # mega slot layout (fp8, per group): [ONES | m0 | g0 | g1 | g2],
        # each [128, GF]. DR rhs for tile t of the pair: view two adjacent
        # slots as [128, 2, GF] and take [:, :, t*NP:(t+1)*NP].
        SLOT = GF

        def pair_ap(mega, s, t):
            v = mega[:, s * SLOT:(s + 2) * SLOT]
            v = v.rearrange("p (a b) -> p a b", a=2)
            return v[:, :, t * NP:(t + 1) * NP]

        # PSUM plan (8 banks of [128,512]f32), all rotations benign:
        #   psY: y [128,1024] bufs=2 (4 banks)  y0/y1/y2 rotation
        #   psB: B [128,512] per-tile bufs=2 (2 banks)
        #   psS: s4 [128,1024] bufs=1 (2 banks)
        with tc.tile_pool(name="mega", bufs=mega_bufs) as megap, \
             tc.tile_pool(name="ysb", bufs=knobs.get("ysb_bufs", 4)) as ysbp, \
             tc.tile_pool(name="psY", bufs=knobs.get("psy_bufs", 2), space="PSUM") as psY, \
             tc.tile_pool(name="psBt", bufs=knobs.get("psb_bufs", 4), space="PSUM") as psBt, \
             tc.tile_pool(name="psS", bufs=knobs.get("pss_bufs", 2), space="PSUM") as psS:

            def st_m0(st):
                g = st["g"]
                mega = megap.tile([H, 5 * SLOT], F8, tag="mega")
                if g < mega_bufs:
                    nc.gpsimd.memset(mega[:, 0:SLOT], 1.0)  # ONES slot, once/buf
                st["mega"] = mega
                for t in range(G):
                    j = g * G + t
                    nc.scalar.activation(
                        mega[:, SLOT + t * NP: SLOT + (t + 1) * NP], pv,
                        ACTF.Prelu, bias=dvf[:, j:j + 1], alpha=0.1)

            def st_pe0(st):
                mega = st["mega"]
                py = psY.tile([H, GF], F32, tag="y")
                pbs = []
                for t in range(G):
                    rhs = pair_ap(mega, 0, t)   # (ONES | m0)
                    nc.tensor.matmul(py[:, t * NP:(t + 1) * NP],
                                     statY[0][:, :].rearrange("p (a b) -> p a b", a=2),
                                     rhs, start=True, stop=True, perf_mode=DRM)
                    pb = psBt.tile([H, NP], F32, tag="b")
                    nc.tensor.matmul(pb,
                                     statB[0][:, :].rearrange("p (a b) -> p a b", a=2),
                                     rhs, start=True, stop=True, perf_mode=DRM)
                    pbs.append(pb)
                st["py"], st["pb"] = py, pbs

            def _evac(st, d):
                py = st["py"]
                y = ysbp.tile([H, GF], BF16, tag="y%d" % d)
                eng = evac_eng[d]
                if eng == "alt":
                    eng = "act" if (st["g"] % alt_mod) == 0 else "dve"
                if eng == "act":
                    nc.scalar.activation(y, py, ACTF.Identity, bias=b_ev[d],
                                         scale=1.0 / (SW * SR))
                else:
                    nc.vector.tensor_scalar(y, py, 1.0 / (SW * SR), b_ev[d],
                                            ALU.mult, ALU.add)
                st["y"] = y

            def _stt(st, d, slot):
                # g_d = (ps_B * 1/SR) * y -> fp8 into mega slot (per tile)
                mega = st["mega"]
                for t in range(G):
                    o = slice(slot * SLOT + t * NP, slot * SLOT + (t + 1) * NP)
                    nc.vector.scalar_tensor_tensor(
                        mega[:, o], st["pb"][t], c_col[d],
                        st["y"][:, t * NP:(t + 1) * NP], ALU.add, ALU.mult)

            def st_ev0(st):
                _evac(st, 0)

            def st_g0(st):
                _stt(st, 0, 2)

            def st_pe1(st):
                mega = st["mega"]
                py = psY.tile([H, GF], F32, tag="y")
                pbs = []
                for t in range(G):
                    r_om = pair_ap(mega, 0, t)   # (ONES | m0)
                    r_mg = pair_ap(mega, 1, t)   # (m0 | g0)
                    o = slice(t * NP, (t + 1) * NP)
                    nc.tensor.matmul(py[:, o], statY[1][:, :].rearrange("p (a b) -> p a b", a=2),
                                     r_om, start=True, stop=False, perf_mode=DRM)
                    nc.tensor.matmul(py[:, o], statYz[1][:, :].rearrange("p (a b) -> p a b", a=2),
                                     r_mg, start=False, stop=True, perf_mode=DRM)
                    pb = psBt.tile([H, NP], F32, tag="b")
                    nc.tensor.matmul(pb, statB[1][:, :].rearrange("p (a b) -> p a b", a=2),
                                     r_om, start=True, stop=False, perf_mode=DRM)
                    nc.tensor.matmul(pb, statBz[1][:, :].rearrange("p (a b) -> p a b", a=2),
                                     r_mg, start=False, stop=True, perf_mode=DRM)
                    pbs.append(pb)
                st["py"], st["pb"] = py, pbs

            def st_ev1(st):
                _evac(st, 1)

            def st_g1(st):
                _stt(st, 1, 3)

            def st_pe2(st):
                mega = st["mega"]
                py = psY.tile([H, GF], F32, tag="y")
                pbs = []
                for t in range(G):
                    r_om = pair_ap(mega, 0, t)   # (ONES | m0)
                    r_gg = pair_ap(mega, 2, t)   # (g0 | g1)
                    o = slice(t * NP, (t + 1) * NP)
                    nc.tensor.matmul(py[:, o], statY[2][:, :].rearrange("p (a b) -> p a b", a=2),
                                     r_om, start=True, stop=False, perf_mode=DRM)
                    nc.tensor.matmul(py[:, o], statY2p[:, :].rearrange("p (a b) -> p a b", a=2),
                                     r_gg, start=False, stop=True, perf_mode=DRM)
                    pb = psBt.tile([H, NP], F32, tag="b")
                    nc.tensor.matmul(pb, statB[2][:, :].rearrange("p (a b) -> p a b", a=2),
                                     r_om, start=True, stop=False, perf_mode=DRM)
                    nc.tensor.matmul(pb, statB2p[:, :].rearrange("p (a b) -> p a b", a=2),
                                     r_gg, start=False, stop=True, perf_mode=DRM)
                    pbs.append(pb)
                st["py"], st["pb"] = py, pbs

            def st_ev2(st):
                _evac(st, 2)

            def st_g2(st):
                _stt(st, 2, 4)

            def st_pe4(st):
                mega = st["mega"]
                p4 = psS.tile([H, GF], F32, tag="s4")
                for t in range(G):
                    r_mg = pair_ap(mega, 1, t)   # (m0 | g0)
                    r_gg2 = pair_ap(mega, 3, t)  # (g1 | g2)
                    o = slice(t * NP, (t + 1) * NP)
                    nc.tensor.matmul(p4[:, o], statV[:, :].rearrange("p (a b) -> p a b", a=2),
                                     r_mg, start=True, stop=False, perf_mode=DRM)
                    nc.tensor.matmul(p4[:, o], statV[:, :].rearrange("p (a b) -> p a b", a=2),
                                     r_gg2, start=False, stop=True, perf_mode=DRM)
                st["p4"] = p4

            def st_zs(st):
                g = st["g"]
                if (g % zs_mod) == 0:
                    zso = ysbp.tile([H, GF], F8, tag="zso")
                    nc.scalar.activation(zso, st["p4"], ACTF.Relu, bias=b1a16,
                                         scale=1.0,
                                         accum_out=sacc[:, g:g + 1])
                else:
                    zso = ysbp.tile([H, GF], BF16, tag="zsv")
                    nc.vector.scalar_tensor_tensor(
                        zso, st["p4"], b1a16, zeros_bf[:, 0:GF],
                        ALU.add, ALU.max, accum_out=sacc[:, g:g + 1])

            stages = [st_m0, st_pe0, st_ev0, st_g0, st_pe1, st_ev1, st_g1,
                      st_pe2, st_ev2, st_g2, st_pe4, st_zs]
            # slot offset of each stage within a group's schedule; groups are
            # spaced SPACING slots apart.
            OFS = knobs.get("ofs", [0, 0, 1, 1, 2, 2, 3, 3, 4, 4, 5, 6])
            SPACING = knobs.get("spacing", 1)
            NST = len(stages)
            live = {}
            total_slots = (NG - 1) * SPACING + OFS[-1] + 1
            # within a slot: oldest group (largest OFS) first; within a group
            # (equal OFS), ascending stage order.
            order = sorted(range(NST), key=lambda s: (-OFS[s], s))
            for slot in range(total_slots):
                for s in order:
                    rem = slot - OFS[s]
                    if rem < 0 or rem % SPACING != 0:
                        continue
                    g = rem // SPACING
                    if g < 0 or g >= NG:
                        continue
                    if s == 0:
                        live[g] = {"g": g}
                    stages[s](live[g])
                    if s == NST - 1:
                        del live[g]

        # ---------------- head (tiny, f32) -------------------------------
        W1b = load("W1b_W", q=gq); W3 = load("W3_W", q=gq)
        Wu1b = load("Wu1b_W"); W5 = load("W5_W")
        W2a_lo = load(None, src=din["W2a_W"][:, 0:H])
        W2a_hi = load(None, src=din["W2a_W"][:, H:2 * H])
        W2b_lo = load(None, src=din["W2b_W"][0:H, :])
        W2b_hi = load(None, src=din["W2b_W"][H:2 * H, :])
        Wu1a_lo = load(None, src=din["Wu1a_W"][0:H, :])
        Wu1a_hi = load(None, src=din["Wu1a_W"][H:2 * H, :])
        Wu_lo = load(None, src=din["Wu_W"][0:H, :])
        Wu_hi = load(None, src=din["Wu_W"][H:2 * H, :])
        Wu_b = load_col(din["Wu_b"]); W1b_b = load_col(din["W1b_b"])
        W2a_b_lo = load_col(din["W2a_b"][0:H]); W2a_b_hi = load_col(din["W2a_b"][H:2 * H])
        W2b_b = load_col(din["W2b_b"]); W3_b = load_col(din["W3_b"])
        Wu1a_b = load_col(din["Wu1a_b"]); Wu1b_b = load_col(din["Wu1b_b"])
        W5_b = load_col(din["W5_b"], n=1)

        with tc.tile_pool(name="head", bufs=1) as hp, \
             tc.tile_pool(name="hpsum", bufs=1, space="PSUM") as hps:
            ps_mu = hps.tile([H, 1], F32, tag="h1")
            nc.tensor.matmul(ps_mu, Wu_lo, u_d, start=True, stop=False)
            nc.tensor.matmul(ps_mu, Wu_hi, u_p, start=False, stop=True)
            m_u = _lrelu_col(nc, hp, ps_mu, Wu_b, 0.01)

            s_raw = hp.tile([H, 1], F32)
            nc.vector.tensor_reduce(s_raw, sacc, mybir.AxisListType.X, ALU.add)
            s_col = hp.tile([H, 1], F32)
            nc.vector.tensor_scalar(s_col, s_raw, 1.0 / SW, None, ALU.mult)

            ps_g1 = hps.tile([H, 1], F32, tag="h1")
            nc.tensor.matmul(ps_g1, W1b, s_col, start=True, stop=True)
            w1bbN = hp.tile([H, 1], F32)
            nc.vector.tensor_scalar(w1bbN, W1b_b, float(NPAIR), None, ALU.mult)
            g1 = hp.tile([H, 1], F32)
            nc.vector.tensor_scalar(g1, ps_g1, w1bbN, None, ALU.add)

            ps_lo = hps.tile([H, 1], F32, tag="h2")
            nc.tensor.matmul(ps_lo, W2a_lo, g1, start=True, stop=True)
            t_lo = _lrelu_col(nc, hp, ps_lo, W2a_b_lo, 0.1)
            ps_hi = hps.tile([H, 1], F32, tag="h3")
            nc.tensor.matmul(ps_hi, W2a_hi, g1, start=True, stop=True)
            t_hi = _lrelu_col(nc, hp, ps_hi, W2a_b_hi, 0.1)

            ps_g2 = hps.tile([H, 1], F32, tag="h4")
            nc.tensor.matmul(ps_g2, W2b_lo, t_lo, start=True, stop=False)
            nc.tensor.matmul(ps_g2, W2b_hi, t_hi, start=False, stop=True)
            g2 = hp.tile([H, 1], F32)
            nc.vector.tensor_scalar(g2, ps_g2, W2b_b, None, ALU.add)

            ps_g3 = hps.tile([H, 1], F32, tag="h5")
            nc.tensor.matmul(ps_g3, W3, g2, start=True, stop=True)
            g3 = _lrelu_col(nc, hp, ps_g3, W3_b, 0.1)

            ps_u = hps.tile([H, 1], F32, tag="h6")
            nc.tensor.matmul(ps_u, Wu1a_lo, m_u, start=True, stop=False)
            nc.tensor.matmul(ps_u, Wu1a_hi, g3, start=False, stop=True)
            h1 = _lrelu_col(nc, hp, ps_u, Wu1a_b, 0.1)

            ps_mu2 = hps.tile([H, 1], F32, tag="h7")
            nc.tensor.matmul(ps_mu2, Wu1b, h1, start=True, stop=True)
            mu = hp.tile([H, 1], F32)
            nc.vector.tensor_scalar(mu, ps_mu2, Wu1b_b, None, ALU.add)

            ps_o = hps.tile([1, 1], F32, tag="h8")
            nc.tensor.matmul(ps_o, W5, mu, start=True, stop=True)
            res = hp.tile([1, 1], F32)
            nc.vector.tensor_scalar(res, ps_o, W5_b, None, ALU.add)
            nc.sync.dma_start(out=dout[:, :], in_=res)


_CACHE = {}


def _get_nc():
    if "nc" not in _CACHE:
        _CACHE["nc"] = build_bass()
    return _CACHE["nc"]


def kernel(**inputs):
    from concourse.bass_utils import run_bass_kernel_spmd

    nc = _get_nc()
    per_core = {"protein_features", "drug_features", "pu_mask", "du_mask"}
    in_maps = []
    for b in range(N_CORES):
        m = {}
        for name in _INPUT_SPECS:
            arr = np.asarray(inputs[name], dtype=np.float32)
            m[name] = np.ascontiguousarray(arr[b]) if name in per_core else arr
        in_maps.append(m)
    res = run_bass_kernel_spmd(nc, in_maps, list(range(N_CORES)))
    out = np.stack([res.results[i]["out"].reshape(1) for i in range(N_CORES)])
    return out.astype(np.float32)


if __name__ == "__main__":
    nc = build_bass()
    print("build ok")
